# revision 5
# baseline (speedup 1.0000x reference)
"""Trainium2 Bass kernel v2 for nn_Evaluate_ZM_55387898250139.

Design:
  Host prep (per core = (batch b, row block r0, 128 rows)):
    - fpad DRAM scratch [(ROWS+125) pair-rows * 256 slots, 256] f16.
      Slot (y, xs) = [P(y,2xs), P(y,2xs+1), P(y,2xs+2), pad40] where
      P(y,x) = [F(:,y,x), F(:,y+1,x)] (72 f16).  One 512B slot holds all
      corners of any bilinear sample with x0 in {2xs, 2xs+1}.
    - idxw [ROWS, 128, 288] int16: per-row dma_gather wrapped indices
      (sample j=m*128+p at partition j%16, col j//16, replicated per 16).
      idx = (y0 - (i-63))*256 + (x0>>1), window rows [i-63, i+62].
    - wf [ROWS, 128, 360] f16: 6 blend weights per sample (216) + own
      feature vector fT (144).
    - oxy [ROWS, 128, 72] f32: transposed offsets for the weighted sum.
  Device per row: 5 dma_gather calls (4608 samples x 512B) -> blend (6
  corners) -> grouped L1 distances (9 pairs) -> min -> sharp softmax ->
  weighted offset outputs.  Vertical outliers (|y0-i|>62, ~1e-4) are
  recomputed exactly on host and patched into the output.
"""
import numpy as np
import ml_dtypes

C = 36
K = 9
GS = 12
NG = 3
HALO = 63           # window rows [i-63, i+62] -> 126 pair-rows indexed


# ----------------------------------------------------------------------------
# Bass kernel builder
# ----------------------------------------------------------------------------

def build_nc(H, W, ROWS, linearize=False):
    import concourse.bacc as bacc
    import concourse.bass as bass
    import concourse.mybir as mybir
    import concourse.tile as tile
    from concourse.masks import make_identity

    F32 = mybir.dt.float32
    F16 = mybir.dt.float16
    I16 = mybir.dt.int16
    ALU = mybir.AluOpType
    AF = mybir.ActivationFunctionType
    AX = mybir.AxisListType

    CH = W // 128
    KC = CH * K                    # 36 samples per partition per row
    SLOTS = W // 2                 # 256 slots per pair-row
    PR = ROWS + 2 * HALO - 1       # 253 pair-rows in scratch
    NIDX = KC * 128                # 4608 samples per row
    WIN = 2 * HALO * SLOTS         # 32256 slots addressable per row

    nc = bacc.Bacc("TRN2", target_bir_lowering=False, debug=False)

    fpad = nc.dram_tensor("fpad", [PR * SLOTS, 256], F16, kind="ExternalInput")
    idxw = nc.dram_tensor("idxw", [ROWS, 32, NIDX // 16], I16, kind="ExternalInput")
    wfb = nc.dram_tensor("wfb", [ROWS, 128, 6 * KC + 4 * C], F16, kind="ExternalInput")
    oxyb = nc.dram_tensor("oxyb", [ROWS, 128, 2 * KC], F32, kind="ExternalInput")
    oxo = nc.dram_tensor("oxo", [ROWS, W], F32, kind="ExternalOutput")
    oyo = nc.dram_tensor("oyo", [ROWS, W], F32, kind="ExternalOutput")

    with tile.TileContext(nc, linearize=linearize) as tc:
        with (
            tc.tile_pool(name="const", bufs=1) as constp,
            tc.tile_pool(name="rowin", bufs=3) as rowin,
            tc.tile_pool(name="gbuf", bufs=2) as gbufp,
            tc.tile_pool(name="mid", bufs=2) as midp,
            tc.tile_pool(name="small", bufs=3) as smallp,
            tc.tile_pool(name="tps", bufs=4, space="PSUM") as tps,
            tc.tile_pool(name="outp", bufs=1) as outp,
        ):
            ident = constp.tile([128, 128], F32)
            make_identity(nc, ident[:])

            OXT = outp.tile([128, CH, ROWS], F32)
            OYT = outp.tile([128, CH, ROWS], F32)

            for i in range(ROWS):
                idx = rowin.tile([32, NIDX // 16], I16, tag="idx")
                nc.sync.dma_start(idx[:], idxw[i])
                wf = rowin.tile([128, 6 * KC + 4 * C], F16, tag="wf")
                nc.sync.dma_start(wf[:], wfb[i])
                oxy = rowin.tile([128, 2 * KC], F32, tag="oxy")
                nc.sync.dma_start(oxy[:], oxyb[i])

                G = gbufp.tile([128, KC, 256], F16, tag="G")
                if i < 2:
                    nc.vector.memset(G[:].rearrange("p a b -> p (a b)"), 0.0)
                src = fpad[i * SLOTS:i * SLOTS + WIN]
                for cc in range(4):
                    nc.gpsimd.dma_gather(
                        G[:, 8 * cc:8 * cc + 8, :], src,
                        idx[:, 64 * cc:64 * cc + 64], 1024, 1024, 256,
                        elem_step=256)
                nc.gpsimd.dma_gather(
                    G[:, 32:36, :], src, idx[:, 256:288], 512, 512, 256,
                    elem_step=256)

                # ---- blend: a[p, m, c] = sum_cr w6[p,m,cr] * G[p,m,cr*36+c]
                w6 = wf[:, :6 * KC].rearrange("p (m cr) -> p m cr", cr=6)
                a = midp.tile([128, KC, C], F16, tag="a")
                t1 = midp.tile([128, KC, C], F16, tag="t1")
                nc.vector.tensor_tensor(
                    a[:], G[:, :, 0:C],
                    w6[:, :, 0][:, :, None].to_broadcast((128, KC, C)),
                    op=ALU.mult)
                for cr in range(1, 6):
                    nc.vector.tensor_tensor(
                        t1[:], G[:, :, cr * C:(cr + 1) * C],
                        w6[:, :, cr][:, :, None].to_broadcast((128, KC, C)),
                        op=ALU.mult)
                    nc.vector.tensor_tensor(a[:], a[:], t1[:], op=ALU.add)

                # ---- d[p, c4, k, v, u, j] = a[p,c4,k,12v+j] - f[p,c4,12u+j]
                fT = wf[:, 6 * KC:].rearrange("p (c4 u j) -> p c4 u j", u=NG, j=GS)
                d = midp.tile([128, KC * NG * NG * GS], F16, tag="d")
                d6 = d[:].rearrange("p (c4 k v u j) -> p c4 k v u j",
                                    c4=CH, k=K, v=NG, u=NG, j=GS)
                a5 = a[:].rearrange("p (c4 k) (v j) -> p c4 k v j", c4=CH, v=NG)
                for v in range(NG):
                    nc.vector.tensor_tensor(
                        d6[:, :, :, v],
                        a5[:, :, :, v][:, :, :, None, :]
                        .to_broadcast((128, CH, K, NG, GS)),
                        fT[:, :, None, :, :].to_broadcast((128, CH, K, NG, GS)),
                        op=ALU.subtract)

                D = midp.tile([128, KC * NG * NG], F32, tag="D")
                nc.vector.tensor_reduce(
                    D[:], d[:].rearrange("p (s j) -> p s j", j=GS),
                    axis=AX.X, op=ALU.add, apply_absolute_value=True)
                Dm = smallp.tile([128, KC], F32, tag="Dm")
                nc.vector.tensor_reduce(
                    Dm[:], D[:].rearrange("p (s q) -> p s q", q=NG * NG),
                    axis=AX.X, op=ALU.min)

                # ---- softmax over k within each chunk (scale -1000/12)
                mmin = smallp.tile([128, CH], F32, tag="mmin")
                nc.vector.tensor_reduce(
                    mmin[:], Dm[:].rearrange("p (c k) -> p c k", k=K),
                    axis=AX.X, op=ALU.min)
                z = smallp.tile([128, KC], F32, tag="z")
                nc.vector.tensor_tensor(
                    z[:].rearrange("p (c k) -> p c k", k=K),
                    Dm[:].rearrange("p (c k) -> p c k", k=K),
                    mmin[:][:, :, None].to_broadcast((128, CH, K)),
                    op=ALU.subtract)
                e = smallp.tile([128, KC], F32, tag="e")
                nc.scalar.activation(e[:], z[:], AF.Exp,
                                     scale=float(np.float32(-1000.0 / GS)))
                ssum = smallp.tile([128, CH], F32, tag="ssum")
                nc.vector.tensor_reduce(
                    ssum[:], e[:].rearrange("p (c k) -> p c k", k=K),
                    axis=AX.X, op=ALU.add)
                rs = smallp.tile([128, CH], F32, tag="rs")
                nc.vector.reciprocal(rs[:], ssum[:])

                for (sl, OT, isx) in ((slice(0, KC), OXT, True),
                                      (slice(KC, 2 * KC), OYT, False)):
                    num = smallp.tile([128, KC], F32, tag="num")
                    nc.vector.tensor_tensor(num[:], e[:], oxy[:, sl], op=ALU.mult)
                    nsum = smallp.tile([128, CH], F32, tag="nsum")
                    nc.vector.tensor_reduce(
                        nsum[:], num[:].rearrange("p (c k) -> p c k", k=K),
                        axis=AX.X, op=ALU.add)
                    nc.vector.tensor_tensor(OT[:, :, i], nsum[:], rs[:],
                                            op=ALU.mult)

            # ---- output: transpose back & store (clip applied on host)
            for (OT, oo) in ((OXT, oxo), (OYT, oyo)):
                OS = outp.tile([ROWS, W], F32, tag="OS")
                for c4 in range(CH):
                    to = tps.tile([ROWS, 128], F32, tag="tp")
                    nc.tensor.transpose(to[:], OT[:, c4, :], ident[:])
                    nc.scalar.activation(OS[:, c4 * 128:(c4 + 1) * 128], to[:],
                                         AF.Copy)
                nc.sync.dma_start(oo[:], OS[:])

    nc.compile()
    return nc


# ----------------------------------------------------------------------------
# Host-side prep
# ----------------------------------------------------------------------------

def _floor_i(px):
    return np.floor(px).astype(np.int32)


def prep_core(features, offset_x, offset_y, H, W, ROWS, b, r0):
    """Build fpad, idxw, wfb, oxyb for one core. Returns (inmap, outlier_cols).

    outlier_cols: bool [ROWS, W] pixels needing host fixup.
    """
    f16 = np.float16
    CH = W // 128
    KC = CH * K
    SLOTS = W // 2
    PR = ROWS + 2 * HALO - 1

    f = features[b]                                    # [C, H, W] f32

    # ---- fpad scratch
    ylo = r0 - HALO                                    # first pair-row (global)
    ys = np.arange(ylo, ylo + PR)
    valid = (ys >= 0) & (ys < H)
    # pairT[li, x, 0:36]=F(:,y,x), [li, x, 36:72]=F(:,y+1,x)
    pairT = np.zeros((PR, W + 2, 72), np.float32)
    yv = ys[valid]
    pairT[valid, :W, 0:C] = np.moveaxis(f[:, yv, :], 0, -1)
    y1v = yv + 1
    ok1 = y1v < H
    sel = np.where(valid)[0][ok1]
    pairT[sel, :W, C:2 * C] = np.moveaxis(f[:, y1v[ok1], :], 0, -1)
    fpad = np.zeros((PR * SLOTS, 256), f16)
    cont = np.concatenate([pairT[:, 0:W:2], pairT[:, 1:W + 1:2],
                           pairT[:, 2:W + 2:2]], axis=2)  # [PR, 256, 216]
    fpad[:, 0:216] = cont.reshape(PR * SLOTS, 216).astype(f16)

    # ---- per-row indices / weights
    i_rows = np.arange(r0, r0 + ROWS, dtype=np.float32)[:, None, None]
    xs_ = np.arange(W, dtype=np.float32)[None, None, :]
    ox = offset_x[b, :, r0:r0 + ROWS, :].transpose(1, 0, 2)  # [ROWS, K, W]
    oy = offset_y[b, :, r0:r0 + ROWS, :].transpose(1, 0, 2)
    px = np.clip(xs_ + ox, 0.0, W - 1).astype(np.float32)
    py = np.clip(i_rows + oy, 0.0, H - 1).astype(np.float32)
    x0 = _floor_i(px)
    y0 = _floor_i(py)
    wx = px - x0
    wy = py - y0

    dy = y0 - np.arange(r0, r0 + ROWS, dtype=np.int32)[:, None, None]
    outlier = (dy < -HALO) | (dy > HALO - 1)           # [ROWS, K, W]
    dyc = np.clip(dy + HALO, 0, 2 * HALO - 1)
    slot = dyc * SLOTS + (x0 >> 1)                     # [ROWS, K, W] in [0,32256)
    slot16 = slot.astype(np.int16)

    par = (x0 & 1).astype(np.float32)
    wc0 = (1.0 - wx) * (1.0 - par)
    wc1 = (1.0 - wx) * par + wx * (1.0 - par)
    wc2 = wx * par
    w6 = np.empty((ROWS, K, W, 6), np.float32)
    w6[..., 0] = wc0 * (1.0 - wy)
    w6[..., 1] = wc0 * wy
    w6[..., 2] = wc1 * (1.0 - wy)
    w6[..., 3] = wc1 * wy
    w6[..., 4] = wc2 * (1.0 - wy)
    w6[..., 5] = wc2 * wy

    # sample m = c4*9+k, p = x%128 ; layouts
    def to_pm(arr):
        # [ROWS, K, W, ...] -> [ROWS, 128, KC, ...] with m=c4*K+k
        a = arr.reshape(ROWS, K, CH, 128, *arr.shape[3:])
        return np.moveaxis(a, (2, 1), (1, 2)).reshape(
            ROWS, KC, 128, *arr.shape[3:]).swapaxes(1, 2)

    # idx wrapped: value of sample (p=16g+q16, m) at [q, m*8+g]
    idx_pm = to_pm(slot16)                             # [ROWS, 128, KC]
    t = idx_pm.reshape(ROWS, 8, 16, KC).transpose(0, 2, 3, 1)  # [ROWS,16,KC,8]
    w16 = t.reshape(ROWS, 16, KC * 8)
    idxw = np.tile(w16, (1, 2, 1)).astype(np.int16)    # [ROWS, 32, 288]

    w6_pm = to_pm(w6).reshape(ROWS, 128, KC * 6)       # [ROWS, 128, 216]
    fT = np.moveaxis(f[:, r0:r0 + ROWS, :], 0, -1)     # [ROWS, W, C]
    fT = fT.reshape(ROWS, CH, 128, C).swapaxes(1, 2).reshape(ROWS, 128, CH * C)
    wfb = np.concatenate([w6_pm, fT], axis=2).astype(f16)  # [ROWS,128,360]

    ox_pm = to_pm(ox.astype(np.float32))               # [ROWS, 128, KC]
    oy_pm = to_pm(oy.astype(np.float32))
    oxyb = np.concatenate([ox_pm, oy_pm], axis=2).astype(np.float32)

    outlier_cols = outlier.any(axis=1)                 # [ROWS, W]
    return ({"fpad": fpad, "idxw": idxw, "wfb": wfb, "oxyb": oxyb},
            outlier_cols)


def fixup_pixels(features, offset_x, offset_y, ox_out, oy_out, b, rows, cols):
    """Exact recompute of flagged pixels (numpy, mirrors reference)."""
    H, W = features.shape[2], features.shape[3]
    f = features[b]                                    # [C, H, W]
    n = rows.shape[0]
    if n == 0:
        return
    ox = offset_x[b][:, rows, cols].astype(np.float32)     # [K, n]
    oy = offset_y[b][:, rows, cols].astype(np.float32)
    px = np.clip(cols[None, :] + ox, 0.0, W - 1)
    py = np.clip(rows[None, :] + oy, 0.0, H - 1)
    x0 = np.floor(px).astype(np.int32)
    y0 = np.floor(py).astype(np.int32)
    wx = (px - x0).astype(np.float32)
    wy = (py - y0).astype(np.float32)
    x0c = np.clip(x0, 0, W - 1)
    x1c = np.clip(x0c + 1, 0, W - 1)
    y0c = np.clip(y0, 0, H - 1)
    y1c = np.clip(y0 + 1, 0, H - 1)
    v00 = f[:, y0c, x0c]                               # [C, K, n]
    v01 = f[:, y0c, x1c]
    v10 = f[:, y1c, x0c]
    v11 = f[:, y1c, x1c]
    a = (v00 * (1 - wx) * (1 - wy) + v01 * wx * (1 - wy)
         + v10 * (1 - wx) * wy + v11 * wx * wy)        # [C, K, n]
    fown = f[:, rows, cols][:, None, :]                # [C, 1, n]
    s = np.empty((NG * NG, K, n), np.float32)
    for u in range(NG):
        for v in range(NG):
            dd = np.abs(fown[u * GS:(u + 1) * GS] - a[v * GS:(v + 1) * GS])
            s[u * NG + v] = -dd.mean(axis=0)
    smax = s.max(axis=0)                               # [K, n]
    z = np.exp(1000.0 * (smax - smax.max(axis=0, keepdims=True)))
    wgt = z / z.sum(axis=0, keepdims=True)
    rf = rows.astype(np.float32)
    cf = cols.astype(np.float32)
    oxv = (ox * wgt).sum(axis=0)
    oyv = (oy * wgt).sum(axis=0)
    oyv = np.clip(oyv + rf, 0.0, H - 1) - rf
    oxv = np.clip(oxv + cf, 0.0, W - 1) - cf
    ox_out[b, 0, rows, cols] = oxv
    oy_out[b, 0, rows, cols] = oyv


# ----------------------------------------------------------------------------
# Entry point
# ----------------------------------------------------------------------------

_NC_CACHE = {}


def kernel(features, offset_x, offset_y, left_x, left_y):
    from concourse import bass_utils

    features = np.asarray(features, np.float32)
    offset_x = np.asarray(offset_x, np.float32)
    offset_y = np.asarray(offset_y, np.float32)
    B, _, H, W = features.shape
    n_cores = 8
    CPB = n_cores // B
    ROWS = H // CPB

    key = (H, W, ROWS)
    if key not in _NC_CACHE:
        _NC_CACHE[key] = build_nc(H, W, ROWS)
    nc = _NC_CACHE[key]

    in_maps, outliers = [], []
    for j in range(n_cores):
        b, r0 = j // CPB, (j % CPB) * ROWS
        m, oc = prep_core(features, offset_x, offset_y, H, W, ROWS, b, r0)
        in_maps.append(m)
        outliers.append(oc)

    res = bass_utils.run_bass_kernel_spmd(nc, in_maps, core_ids=list(range(n_cores)))

    ox = np.zeros((B, 1, H, W), np.float32)
    oy = np.zeros((B, 1, H, W), np.float32)
    xs_ = np.arange(W, dtype=np.float32)[None, :]
    for j, r in enumerate(res.results):
        b, r0 = j // CPB, (j % CPB) * ROWS
        rr = np.arange(r0, r0 + ROWS, dtype=np.float32)[:, None]
        oxv = np.clip(r["oxo"] + xs_, 0.0, W - 1) - xs_
        oyv = np.clip(r["oyo"] + rr, 0.0, H - 1) - rr
        ox[b, 0, r0:r0 + ROWS] = oxv
        oy[b, 0, r0:r0 + ROWS] = oyv

    for j in range(n_cores):
        b, r0 = j // CPB, (j % CPB) * ROWS
        rws, cls = np.nonzero(outliers[j])
        fixup_pixels(features, offset_x, offset_y, ox, oy, b, rws + r0, cls)
    return ox, oy
